# revision 26
# baseline (speedup 1.0000x reference)
"""Multi-head attention Trainium2 kernel (8 NeuronCores).

Problem: B=2, T=2048, E=1024, H=16, D=64 multi-head attention
    q/k/v = einsum('bte,hed->bhtd', x, W{q,k,v})
    out   = softmax(q k^T / sqrt(D)) v, heads concat, @ Wo, + x

Sharding: data-parallel over batch (2 groups of 4 cores) x tensor-parallel
over heads (4 heads per core). Each core computes, for its batch b and its
4 heads, the partial output  partial = concat_heads(attn) @ Wo[head rows].
The host sums the 4 partials per batch and adds the residual x.

Device layout notes:
  - All matmul inputs are bf16 (fp32 PSUM accumulation); exp runs on the
    scalar engine in fp32 reading scores straight from PSUM.
  - Scores are computed transposed, S^T[T', t], so that P^T = exp(S^T)
    lands directly in the layout the PV matmul needs as its moving
    operand (contraction over T' on partitions).
  - The softmax denominator rides along the PV matmul: the stationary
    operand is [V_h | 1] (65 columns), so PSUM rows 0..63 = (P V_h)^T and
    row 64 = sum_T' P = the denominator. No separate denominator matmuls.
  - Normalization multiplies by a DMA-broadcast reciprocal row (the
    compute engines cannot broadcast along partitions; DMA can).
  - exp is not max-subtracted: scores/8 lie in roughly [-10, 10] for this
    problem family, far inside fp32 exp range.
  - PSUM: scores ring 2x[128,2,512] (4 banks) + proj/out-proj ring
    2x[128,1,512] (2 banks) + 2 PV accumulators [65,512] (2 banks) = 8.
"""

import contextlib
import ctypes
import os
import sys
import types

import numpy as np
import ml_dtypes

B, T, E, H = 2, 2048, 1024, 16
D = E // H          # 64
NCORES = 8
DP = 2              # batch groups
TPC = NCORES // DP  # cores per batch group
HLOC = H // TPC     # heads per core = 4
CLOC = HLOC * D     # local concat width = 256

_cached_nc = None
LAST_EXEC_NS = None


def _ensure_ntff_hook():
    """bass_utils' trace path imports antenv.axon_hooks, which is absent in
    this image. Recreate it (registry + ctypes NTFF driver) so profiled runs
    don't crash; no-op if the module already exists."""
    try:
        import antenv.axon_hooks  # noqa: F401
        return
    except ImportError:
        pass
    try:
        import antenv
    except ImportError:
        return

    mod = types.ModuleType("antenv.axon_hooks")
    _state = {"hook": None}
    mod.set_axon_ntff_profile_hook = lambda h: _state.__setitem__("hook", h)
    mod.get_axon_ntff_profile_hook = lambda: _state["hook"]
    sys.modules["antenv.axon_hooks"] = mod
    antenv.axon_hooks = mod

    so_path = "/opt/axon/libaxon_pjrt.so"
    if not os.path.exists(so_path):
        return
    try:
        lib = ctypes.CDLL(so_path)
    except OSError:
        return
    if not hasattr(lib, "axon_start_nrt_profile"):
        return
    lib.axon_start_nrt_profile.argtypes = [
        ctypes.POINTER(ctypes.c_int64),
        ctypes.c_size_t,
    ]
    lib.axon_start_nrt_profile.restype = ctypes.c_int64
    lib.axon_stop_nrt_profile.argtypes = [ctypes.c_char_p]
    lib.axon_stop_nrt_profile.restype = ctypes.c_int64

    @contextlib.contextmanager
    def _hook(output_dir, device_ids):
        import jax

        jax.devices()
        if device_ids:
            ids = (ctypes.c_int64 * len(device_ids))(*device_ids)
            rc = lib.axon_start_nrt_profile(ids, len(device_ids))
        else:
            rc = lib.axon_start_nrt_profile(None, 0)
        if rc != 0:
            raise RuntimeError(f"axon_start_nrt_profile rc={rc}")
        try:
            yield
        finally:
            n = lib.axon_stop_nrt_profile(str(output_dir).encode())
            print(f"ntff profile: {n} file(s) -> {output_dir}", file=sys.stderr)

    mod.set_axon_ntff_profile_hook(_hook)


def _build_program():
    import concourse.mybir as mybir
    import concourse.tile as tile
    from concourse import bacc
    from concourse.tile_rust import add_dep_helper

    def _inst(bi):
        return bi.ins if hasattr(bi, "ins") else bi

    f32 = mybir.dt.float32
    bf16 = mybir.dt.bfloat16
    AF = mybir.ActivationFunctionType

    nc = bacc.Bacc("TRN2", target_bir_lowering=False, debug=False,
                   num_devices=NCORES)

    xT = nc.declare_dram_parameter("xT", [E, T], bf16, isOutput=False)
    wq = nc.declare_dram_parameter("wq", [E, CLOC], bf16, isOutput=False)
    wk = nc.declare_dram_parameter("wk", [E, CLOC], bf16, isOutput=False)
    wv = nc.declare_dram_parameter("wv", [E, CLOC], bf16, isOutput=False)
    wo = nc.declare_dram_parameter("wo", [CLOC, E], bf16, isOutput=False)
    out = nc.declare_dram_parameter("out", [T, E], f32, isOutput=True)

    KC = E // 128        # 8 contraction chunks for the projections
    NT = T // 128        # 16 T'-tiles (key rows per tile)
    NTC = T // 512       # 4 t-chunks (query columns per chunk)
    NPAIR = HLOC // 2    # 2 head pairs

    with tile.TileContext(nc) as tc:
        with (
            tc.tile_pool(name="persist", bufs=1) as persist,
            tc.tile_pool(name="spool", bufs=2, space="PSUM") as spool,
            tc.tile_pool(name="ppool", bufs=2, space="PSUM") as ppool,
            tc.tile_pool(name="pvpool", bufs=1, space="PSUM") as pvpool,
            tc.tile_pool(name="ptile", bufs=6) as ptile,
            tc.tile_pool(name="small", bufs=6) as small,
            tc.tile_pool(name="dscratch", bufs=8, space="DRAM") as dscratch,
        ):
            # ---- stage inputs in SBUF (all bf16) ----
            # Ordered so the very first projection chunk (pair-0 K and Q on
            # t-chunk 0) unblocks as early as possible: its weights halves
            # and the chunk-0 slice of x^T go first, then wv (needed by the
            # jit V projection inside the first attention chunk), then the
            # rest in consumption order. sync/gpsimd alternate to spread the
            # 16 DMA queues; the scalar engine issues nothing so exp starts
            # unimpeded.
            # Few LARGE transfers: one dma_start spreads its descriptors
            # over all 16 SDMA queues, so consolidation keeps full HBM
            # bandwidth while slashing the ~0.5us-per-issue sequencer cost
            # that used to pace the ramp.
            xT_sb = persist.tile([128, KC, T], bf16)
            wq_sb = persist.tile([128, KC, CLOC], bf16)
            wk_sb = persist.tile([128, KC, CLOC], bf16)
            wv_sb = persist.tile([128, KC, CLOC], bf16)
            xT_r = xT.ap().rearrange("(a p) t -> p a t", p=128)
            wq_r = wq.ap().rearrange("(a p) c -> p a c", p=128)
            wk_r = wk.ap().rearrange("(a p) c -> p a c", p=128)
            wv_r = wv.ap().rearrange("(a p) c -> p a c", p=128)
            nc.sync.dma_start(out=wk_sb[:, :, 0:128], in_=wk_r[:, :, 0:128])
            nc.gpsimd.dma_start(out=wq_sb[:, :, 0:128], in_=wq_r[:, :, 0:128])
            for h in (slice(0, 64), slice(64, 128)):
                nc.sync.dma_start(
                    out=xT_sb[h, :, 0:512], in_=xT_r[h, :, 0:512]
                )
            nc.gpsimd.dma_start(out=wv_sb[:], in_=wv_r[:])
            for h in (slice(0, 64), slice(64, 128)):
                nc.sync.dma_start(
                    out=xT_sb[h, :, 512:1024], in_=xT_r[h, :, 512:1024]
                )
            for h in (slice(0, 64), slice(64, 128)):
                nc.sync.dma_start(
                    out=xT_sb[h, :, 1024:1536], in_=xT_r[h, :, 1024:1536]
                )
                nc.sync.dma_start(
                    out=xT_sb[h, :, 1536:T], in_=xT_r[h, :, 1536:T]
                )
            nc.gpsimd.dma_start(
                out=wk_sb[:, :, 128:256], in_=wk_r[:, :, 128:256]
            )
            nc.gpsimd.dma_start(
                out=wq_sb[:, :, 128:256], in_=wq_r[:, :, 128:256]
            )
            # Wo rows for head pair pp live at partitions 0..127 of plane pp.
            wo_sb = persist.tile([128, HLOC // 2, E], bf16)
            wo_r = wo.ap().rearrange("(pp r) e -> r pp e", r=128)
            for pp in range(HLOC // 2):
                nc.gpsimd.dma_start(out=wo_sb[:, pp, :], in_=wo_r[:, pp, :])

            # ---- projections ----
            # Q^T, K^T: [CLOC, T] with head-local d on partitions
            # (M-group mg holds heads 2mg, 2mg+1).
            qT_sb = persist.tile([128, NPAIR, T], bf16)
            kT_sb = persist.tile([128, NPAIR, T], bf16)
            # V per t-tile in natural layout, with a ones column appended per
            # head: vp_sb[:, tt, h, 0:64] = V_h rows, [:, tt, h, 64] = 1.
            # PV stationary [V_h | 1] then yields the softmax denominator in
            # PSUM row 64 for free.
            vp_sb = persist.tile([128, NT, HLOC, D + 1], bf16)
            nc.vector.memset(vp_sb[:, :, :, D : D + 1], 1.0)

            def proj_chunk(w_sb, dst, mg, c):
                """One 512-wide t-chunk of one head pair's projection."""
                ps = ppool.tile([128, 1, 512], f32, tag="pp")
                for kc in range(KC):
                    nc.tensor.matmul(
                        ps[:, 0, :],
                        lhsT=w_sb[:, kc, mg * 128 : (mg + 1) * 128],
                        rhs=xT_sb[:, kc, c * 512 : (c + 1) * 512],
                        start=(kc == 0),
                        stop=(kc == KC - 1),
                    )
                nc.vector.tensor_copy(
                    out=dst[:, mg, c * 512 : (c + 1) * 512], in_=ps[:, 0, :]
                )

            def proj_chunk_units(w_sb, dst, mg, c):
                """Generator form of proj_chunk: yields after every matmul
                so the filler driver can meter it out ~2 matmuls per
                attention group (the PE slack under the exp-paced loop)."""
                ps = ppool.tile([128, 1, 512], f32, tag="pp")
                for kc in range(KC):
                    nc.tensor.matmul(
                        ps[:, 0, :],
                        lhsT=w_sb[:, kc, mg * 128 : (mg + 1) * 128],
                        rhs=xT_sb[:, kc, c * 512 : (c + 1) * 512],
                        start=(kc == 0),
                        stop=(kc == KC - 1),
                    )
                    yield
                nc.vector.tensor_copy(
                    out=dst[:, mg, c * 512 : (c + 1) * 512], in_=ps[:, 0, :]
                )
                yield

            def out_proj_units(tt):
                """Generator: partial = headsN^T @ Wo_loc for t-tile tt."""
                ksl = slice(tt * 128, (tt + 1) * 128)
                psa = ppool.tile([128, 1, 512], f32, tag="pp")
                psb = ppool.tile([128, 1, 512], f32, tag="pp")
                for pp in range(NPAIR):
                    nc.tensor.matmul(
                        psa[:, 0, :],
                        lhsT=headsN[:, pp, ksl],
                        rhs=wo_sb[:, pp, 0:512],
                        start=(pp == 0),
                        stop=(pp == NPAIR - 1),
                    )
                    nc.tensor.matmul(
                        psb[:, 0, :],
                        lhsT=headsN[:, pp, ksl],
                        rhs=wo_sb[:, pp, 512:1024],
                        start=(pp == 0),
                        stop=(pp == NPAIR - 1),
                    )
                    yield
                stg = small.tile([128, 1024], f32, tag="ostg")
                nc.vector.tensor_copy(out=stg[:, 0:512], in_=psa[:, 0, :])
                nc.vector.tensor_copy(out=stg[:, 512:1024], in_=psb[:, 0, :])
                nc.sync.dma_start(out=out.ap()[ksl, :], in_=stg[:])
                yield

            class Filler:
                """Meters queued generators out in ~1-matmul units; the
                emission position of each unit sets its scheduler priority
                so projection/out-proj matmuls slot into the PE slack of
                the exp-paced attention loop instead of forming dense
                blocks that starve the scalar engine.

                CRITICAL: Tile builds the dependency graph in emission
                order, so a consumer emitted before its producer reads
                garbage. require(gen) force-drains up to a generator and
                must be called before emitting anything that reads its
                output."""

                def __init__(self):
                    self.gens = []

                def add(self, gen):
                    self.gens.append(gen)
                    return gen

                def fill(self, n):
                    while n > 0 and self.gens:
                        try:
                            next(self.gens[0])
                            n -= 1
                        except StopIteration:
                            self.gens.pop(0)

                def require(self, gen):
                    while gen in self.gens:
                        self.fill(16)

                def drain(self):
                    while self.gens:
                        self.fill(64)

            def v_proj_pair(tt0):
                # V tiles tt0, tt0+1 in natural [t, c] layout via x^T as the
                # stationary side; both share one ppool tile and one eviction
                ps = ppool.tile([128, 1, 512], f32, tag="pp")
                for half in range(2):
                    tt = tt0 + half
                    for kc in range(KC):
                        nc.tensor.matmul(
                            ps[:, 0, half * CLOC : (half + 1) * CLOC],
                            lhsT=xT_sb[:, kc, tt * 128 : (tt + 1) * 128],
                            rhs=wv_sb[:, kc, :],
                            start=(kc == 0),
                            stop=(kc == KC - 1),
                        )
                nc.vector.tensor_copy(
                    out=vp_sb[:, tt0 : tt0 + 2, :, 0:D],
                    in_=ps[:, 0, :].rearrange(
                        "p (a h d) -> p a h d", a=2, h=HLOC
                    ),
                )

            # headsN[c_lo, pair, t]: plane `pair` holds heads 2p (partitions
            # 0..63) and 2p+1 (64..127) — ready as out-proj stationary tiles.
            headsN = persist.tile([128, NPAIR, T], bf16)

            def attention(pair, tcn, jit_vproj=False, fill=None, rate=2,
                          last=False):
                h0, h1 = 2 * pair, 2 * pair + 1
                tsl = slice(tcn * 512, (tcn + 1) * 512)
                # PV accumulators: one bank per head, rows 0..63 = (P V_h)^T,
                # row 64 = softmax denominator.
                pva = pvpool.tile([D + 1, 512], f32, tag="pva")
                pvb = pvpool.tile([D + 1, 512], f32, tag="pvb")
                # Process T'-tiles in pairs, batching same-shaped matmuls
                # back-to-back — the PE only pipelines (drain under next
                # fill) within runs of same-configuration instructions.
                for g in range(NT // 2):
                    st = (g == 0)
                    sp = (g == NT // 2 - 1)
                    pss, pts = [], []
                    last_s = None
                    for i in range(2):
                        tt = 2 * g + i
                        ksl = slice(tt * 128, (tt + 1) * 128)
                        ps_s = spool.tile([128, 2, 512], f32, tag="sc")
                        pss.append(ps_s)
                        # S^T for both heads (row-packed on the PE)
                        nc.tensor.matmul(
                            ps_s[:, 0, :],
                            lhsT=kT_sb[0:64, pair, ksl],
                            rhs=qT_sb[0:64, pair, tsl],
                            tile_position=(0, 0),
                        )
                        last_s = nc.tensor.matmul(
                            ps_s[:, 1, :],
                            lhsT=kT_sb[64:128, pair, ksl],
                            rhs=qT_sb[64:128, pair, tsl],
                            tile_position=(64, 0),
                        )
                    for i in range(2):
                        pt = ptile.tile([128, 2, 512], bf16, tag="pt")
                        pts.append(pt)
                        nc.scalar.activation(
                            out=pt[:], in_=pss[i][:], func=AF.Exp,
                            scale=0.125,
                        )
                    if jit_vproj:
                        # first consumer of these V tiles: project them
                        # now so the PE fills exp-wait slack
                        v_proj_pair(2 * g)
                    if fill is not None:
                        fill(rate)
                    for i in range(2):
                        tt = 2 * g + i
                        mm = nc.tensor.matmul(
                            pva[:],
                            lhsT=vp_sb[:, tt, h0, :],
                            rhs=pts[i][:, 0, :],
                            start=(st and i == 0), stop=(sp and i == 1),
                        )
                        if i == 0:
                            # keep the two scores groups adjacent on the
                            # PE (same-config runs pipeline; interleaved
                            # configs pay the full isolated matmul cost)
                            add_dep_helper(
                                _inst(mm), _inst(last_s),
                                reason="batch scores before pv",
                            )
                        nc.tensor.matmul(
                            pvb[:],
                            lhsT=vp_sb[:, tt, h1, :],
                            rhs=pts[i][:, 1, :],
                            start=(st and i == 0), stop=(sp and i == 1),
                        )
                # evacuate PSUM quickly so the next chunk's PV can start.
                # h1's rows land on partitions 0..64 (PSUM out starts at the
                # tile base) but must end up on partitions 64..127 of headsN;
                # compute engines can't cross partitions, so DMA-shift the
                # staged copy while the denominator broadcast is in flight.
                stga = small.tile([D + 1, 512], f32, tag="stga")
                nc.vector.tensor_copy(out=stga[:], in_=pva[:])
                stgb = small.tile([D + 1, 512], f32, tag="stgb")
                nc.vector.tensor_copy(out=stgb[:], in_=pvb[:])
                stgb_hi = small.tile([128, 512], f32, tag="stgbh")
                nc.gpsimd.dma_start(
                    out=stgb_hi[64:128, :], in_=stgb[0:D, :]
                )
                # broadcast each head's denominator row across 64 partitions
                # via a DRAM bounce, then one reciprocal + two multiplies.
                # Mid-kernel the bounce latency hides under the next phase;
                # in the last phase it is the critical path, so spread the
                # four transfers over four otherwise-idle engine queues.
                den = small.tile([128, 512], f32, tag="den")
                dsc_eng = (nc.sync, nc.scalar if last else nc.sync)
                bc_eng = (nc.gpsimd, nc.scalar if last else nc.gpsimd)
                for j, stg in enumerate((stga, stgb)):
                    dsc = dscratch.tile([1, 512], f32, tag="dsc")
                    dsc_eng[j].dma_start(out=dsc[:], in_=stg[D : D + 1, :])
                    bc_eng[j].dma_start(
                        out=den[j * 64 : (j + 1) * 64, :],
                        in_=dsc[:].to_broadcast([64, 512]),
                    )
                rec = small.tile([128, 512], f32, tag="recb")
                nc.vector.reciprocal_approx_fast(out=rec[:], in_=den[:])
                nc.vector.tensor_mul(
                    out=headsN[0:64, pair, tsl], in0=stga[0:D, :],
                    in1=rec[0:64, :],
                )
                nc.vector.tensor_mul(
                    out=headsN[64:128, pair, tsl], in0=stgb_hi[64:128, :],
                    in1=rec[64:128, :],
                )

            # Emission order IS program order (Tile tracks deps in trace
            # order) and acts as the scheduler's priority tiebreak among
            # ready instructions. The attention chain is exp-paced, so the
            # PE has ~2 spare matmul slots per group; the filler meters the
            # remaining projections and the out-projection through those
            # slots instead of letting them form dense blocks that would
            # starve the scalar engine (or sit in dead phases at the ends).
            filler = Filler()
            proj_chunk(wk_sb, kT_sb, 0, 0)
            proj_chunk(wq_sb, qT_sb, 0, 0)
            proj_chunk(wk_sb, kT_sb, 0, 1)
            # kT chunks 2,3 ride as rate-4 filler inside att(0,0): their
            # evictions are emitted (unit 9 / 18) before the scores of
            # groups 4 / 6 that read them (fill counts 16 / 24).
            filler.add(proj_chunk_units(wk_sb, kT_sb, 0, 2))
            filler.add(proj_chunk_units(wk_sb, kT_sb, 0, 3))
            q01 = filler.add(proj_chunk_units(wq_sb, qT_sb, 0, 1))
            attention(0, 0, jit_vproj=True, fill=filler.fill, rate=4)
            filler.require(q01)
            q02 = filler.add(proj_chunk_units(wq_sb, qT_sb, 0, 2))
            q03 = filler.add(proj_chunk_units(wq_sb, qT_sb, 0, 3))
            for c in range(NTC):
                filler.add(proj_chunk_units(wk_sb, kT_sb, 1, c))
            q10 = filler.add(proj_chunk_units(wq_sb, qT_sb, 1, 0))
            q11 = filler.add(proj_chunk_units(wq_sb, qT_sb, 1, 1))
            attention(0, 1, fill=filler.fill, rate=3)
            filler.require(q02)
            attention(0, 2, fill=filler.fill, rate=3)
            filler.require(q03)
            attention(0, 3, fill=filler.fill, rate=3)
            filler.require(q10)
            q12 = filler.add(proj_chunk_units(wq_sb, qT_sb, 1, 2))
            attention(1, 0, fill=filler.fill)
            filler.require(q11)
            q13 = filler.add(proj_chunk_units(wq_sb, qT_sb, 1, 3))
            for tt in range(0, 4):
                filler.add(out_proj_units(tt))
            attention(1, 1, fill=filler.fill)
            filler.require(q12)
            for tt in range(4, 8):
                filler.add(out_proj_units(tt))
            attention(1, 2, fill=filler.fill)
            filler.require(q13)
            for tt in range(8, 12):
                filler.add(out_proj_units(tt))
            attention(1, 3, fill=filler.fill, last=True)
            for tt in range(12, 16):
                filler.add(out_proj_units(tt))
            filler.drain()

    nc.compile()
    return nc


def _get_program():
    global _cached_nc
    if _cached_nc is None:
        _cached_nc = _build_program()
    return _cached_nc


def kernel(x, Wq, Wk, Wv, Wo):
    global LAST_EXEC_NS
    _ensure_ntff_hook()
    from concourse.bass_utils import run_bass_kernel_spmd

    nc = _get_program()
    bf16 = ml_dtypes.bfloat16

    x = np.asarray(x, dtype=np.float32)
    in_maps = []
    for c in range(NCORES):
        b = c // TPC
        hs = (c % TPC) * HLOC
        xT_c = np.ascontiguousarray(x[b].T).astype(bf16)
        # [HLOC, E, D] -> [E, HLOC*D]
        wq_c = np.ascontiguousarray(
            np.asarray(Wq)[hs : hs + HLOC].transpose(1, 0, 2).reshape(E, CLOC)
        ).astype(bf16)
        wk_c = np.ascontiguousarray(
            np.asarray(Wk)[hs : hs + HLOC].transpose(1, 0, 2).reshape(E, CLOC)
        ).astype(bf16)
        wv_c = np.ascontiguousarray(
            np.asarray(Wv)[hs : hs + HLOC].transpose(1, 0, 2).reshape(E, CLOC)
        ).astype(bf16)
        wo_c = np.ascontiguousarray(
            np.asarray(Wo)[hs * D : (hs + HLOC) * D, :]
        ).astype(bf16)
        in_maps.append(
            {"xT": xT_c, "wq": wq_c, "wk": wk_c, "wv": wv_c, "wo": wo_c}
        )

    trace = bool(os.environ.get("KERNEL_TRACE"))
    res = run_bass_kernel_spmd(nc, in_maps, list(range(NCORES)), trace=trace)
    LAST_EXEC_NS = res.exec_time_ns

    out = np.empty((B, T, E), dtype=np.float32)
    for b in range(B):
        acc = x[b].copy()
        for g in range(TPC):
            acc += res.results[b * TPC + g]["out"]
        out[b] = acc
    return out


# revision 32
# speedup vs baseline: 1.1639x; 1.1639x over previous
"""Multi-head attention Trainium2 kernel (8 NeuronCores).

Problem: B=2, T=2048, E=1024, H=16, D=64 multi-head attention
    q/k/v = einsum('bte,hed->bhtd', x, W{q,k,v})
    out   = softmax(q k^T / sqrt(D)) v, heads concat, @ Wo, + x

Sharding: data-parallel over batch (2 groups of 4 cores) x tensor-parallel
over heads (4 heads per core). Each core computes, for its batch b and its
4 heads, the partial output  partial = concat_heads(attn) @ Wo[head rows].
The host sums the 4 partials per batch and adds the residual x.

Device layout notes:
  - All matmul inputs are bf16 (fp32 PSUM accumulation); exp runs on the
    scalar engine in fp32 reading scores straight from PSUM.
  - Scores are computed transposed, S^T[T', t], so that P^T = exp(S^T)
    lands directly in the layout the PV matmul needs as its moving
    operand (contraction over T' on partitions).
  - The softmax denominator rides along the PV matmul: the stationary
    operand is [V_h | 1] (65 columns), so PSUM rows 0..63 = (P V_h)^T and
    row 64 = sum_T' P = the denominator. No separate denominator matmuls.
  - Normalization multiplies by a DMA-broadcast reciprocal row (the
    compute engines cannot broadcast along partitions; DMA can).
  - exp is not max-subtracted: scores/8 lie in roughly [-10, 10] for this
    problem family, far inside fp32 exp range.
  - PSUM: scores ring 2x[128,2,512] (4 banks) + proj/out-proj ring
    2x[128,1,512] (2 banks) + 2 PV accumulators [65,512] (2 banks) = 8.
"""

import contextlib
import ctypes
import os
import sys
import types

import numpy as np
import ml_dtypes

B, T, E, H = 2, 2048, 1024, 16
D = E // H          # 64
NCORES = 8
DP = 2              # batch groups
TPC = NCORES // DP  # cores per batch group
HLOC = H // TPC     # heads per core = 4
CLOC = HLOC * D     # local concat width = 256

_cached_nc = None
LAST_EXEC_NS = None


def _ensure_ntff_hook():
    """bass_utils' trace path imports antenv.axon_hooks, which is absent in
    this image. Recreate it (registry + ctypes NTFF driver) so profiled runs
    don't crash; no-op if the module already exists."""
    try:
        import antenv.axon_hooks  # noqa: F401
        return
    except ImportError:
        pass
    try:
        import antenv
    except ImportError:
        return

    mod = types.ModuleType("antenv.axon_hooks")
    _state = {"hook": None}
    mod.set_axon_ntff_profile_hook = lambda h: _state.__setitem__("hook", h)
    mod.get_axon_ntff_profile_hook = lambda: _state["hook"]
    sys.modules["antenv.axon_hooks"] = mod
    antenv.axon_hooks = mod

    so_path = "/opt/axon/libaxon_pjrt.so"
    if not os.path.exists(so_path):
        return
    try:
        lib = ctypes.CDLL(so_path)
    except OSError:
        return
    if not hasattr(lib, "axon_start_nrt_profile"):
        return
    lib.axon_start_nrt_profile.argtypes = [
        ctypes.POINTER(ctypes.c_int64),
        ctypes.c_size_t,
    ]
    lib.axon_start_nrt_profile.restype = ctypes.c_int64
    lib.axon_stop_nrt_profile.argtypes = [ctypes.c_char_p]
    lib.axon_stop_nrt_profile.restype = ctypes.c_int64

    @contextlib.contextmanager
    def _hook(output_dir, device_ids):
        import jax

        jax.devices()
        if device_ids:
            ids = (ctypes.c_int64 * len(device_ids))(*device_ids)
            rc = lib.axon_start_nrt_profile(ids, len(device_ids))
        else:
            rc = lib.axon_start_nrt_profile(None, 0)
        if rc != 0:
            raise RuntimeError(f"axon_start_nrt_profile rc={rc}")
        try:
            yield
        finally:
            n = lib.axon_stop_nrt_profile(str(output_dir).encode())
            print(f"ntff profile: {n} file(s) -> {output_dir}", file=sys.stderr)

    mod.set_axon_ntff_profile_hook(_hook)


def _build_program():
    import concourse.mybir as mybir
    import concourse.tile as tile
    from concourse import bacc
    from concourse.tile_rust import add_dep_helper

    def _inst(bi):
        return bi.ins if hasattr(bi, "ins") else bi

    f32 = mybir.dt.float32
    bf16 = mybir.dt.bfloat16
    AF = mybir.ActivationFunctionType

    nc = bacc.Bacc("TRN2", target_bir_lowering=False, debug=False,
                   num_devices=NCORES)

    xT = nc.declare_dram_parameter("xT", [E, T], bf16, isOutput=False)
    wq = nc.declare_dram_parameter("wq", [E, CLOC], bf16, isOutput=False)
    wk = nc.declare_dram_parameter("wk", [E, CLOC], bf16, isOutput=False)
    wv = nc.declare_dram_parameter("wv", [E, CLOC], bf16, isOutput=False)
    wo = nc.declare_dram_parameter("wo", [CLOC, E], bf16, isOutput=False)
    out = nc.declare_dram_parameter("out", [T, E], f32, isOutput=True)

    KC = E // 128        # 8 contraction chunks for the projections
    NT = T // 128        # 16 T'-tiles (key rows per tile)
    NTC = T // 512       # 4 t-chunks (query columns per chunk)
    NPAIR = HLOC // 2    # 2 head pairs

    with tile.TileContext(nc) as tc:
        with (
            tc.tile_pool(name="persist", bufs=1) as persist,
            tc.tile_pool(name="spool", bufs=2, space="PSUM") as spool,
            tc.tile_pool(name="ppool", bufs=2, space="PSUM") as ppool,
            tc.tile_pool(name="pvpool", bufs=1, space="PSUM") as pvpool,
            tc.tile_pool(name="ptile", bufs=6) as ptile,
            tc.tile_pool(name="small", bufs=6) as small,
            tc.tile_pool(name="dscratch", bufs=8, space="DRAM") as dscratch,
        ):
            # ---- stage inputs in SBUF (all bf16) ----
            # Ordered so the very first projection chunk (pair-0 K and Q on
            # t-chunk 0) unblocks as early as possible: its weights halves
            # and the chunk-0 slice of x^T go first, then wv (needed by the
            # jit V projection inside the first attention chunk), then the
            # rest in consumption order. sync/gpsimd alternate to spread the
            # 16 DMA queues; the scalar engine issues nothing so exp starts
            # unimpeded.
            # Few LARGE transfers: one dma_start spreads its descriptors
            # over all 16 SDMA queues, so consolidation keeps full HBM
            # bandwidth while slashing the ~0.5us-per-issue sequencer cost
            # that used to pace the ramp.
            xT_sb = persist.tile([128, KC, T], bf16)
            wq_sb = persist.tile([128, KC, CLOC], bf16)
            wk_sb = persist.tile([128, KC, CLOC], bf16)
            wv_sb = persist.tile([128, KC, CLOC], bf16)
            xT_r = xT.ap().rearrange("(a p) t -> p a t", p=128)
            wq_r = wq.ap().rearrange("(a p) c -> p a c", p=128)
            wk_r = wk.ap().rearrange("(a p) c -> p a c", p=128)
            wv_r = wv.ap().rearrange("(a p) c -> p a c", p=128)
            # First-needed pieces go per-kc on TWO alternating queues (sync
            # for wk, the otherwise-idle scalar queue for x^T) so the first
            # projection group is paced by ~0.5us/kc of landing data rather
            # than one sequencer's issue rate.
            for kc in range(KC):
                nc.sync.dma_start(
                    out=wk_sb[:, kc, 0:128], in_=wk_r[:, kc, 0:128]
                )
                nc.scalar.dma_start(
                    out=xT_sb[:, kc, 0:512], in_=xT_r[:, kc, 0:512]
                )
            nc.gpsimd.dma_start(out=wq_sb[:, :, 0:128], in_=wq_r[:, :, 0:128])
            nc.gpsimd.dma_start(out=wv_sb[:], in_=wv_r[:])
            for h in (slice(0, 64), slice(64, 128)):
                nc.sync.dma_start(
                    out=xT_sb[h, :, 512:1024], in_=xT_r[h, :, 512:1024]
                )
            for h in (slice(0, 64), slice(64, 128)):
                nc.sync.dma_start(
                    out=xT_sb[h, :, 1024:1536], in_=xT_r[h, :, 1024:1536]
                )
                nc.sync.dma_start(
                    out=xT_sb[h, :, 1536:T], in_=xT_r[h, :, 1536:T]
                )
            nc.gpsimd.dma_start(
                out=wk_sb[:, :, 128:256], in_=wk_r[:, :, 128:256]
            )
            nc.gpsimd.dma_start(
                out=wq_sb[:, :, 128:256], in_=wq_r[:, :, 128:256]
            )
            # Wo rows for head pair pp live at partitions 0..127 of plane pp.
            wo_sb = persist.tile([128, HLOC // 2, E], bf16)
            wo_r = wo.ap().rearrange("(pp r) e -> r pp e", r=128)
            for pp in range(HLOC // 2):
                nc.gpsimd.dma_start(out=wo_sb[:, pp, :], in_=wo_r[:, pp, :])

            # ---- projections ----
            # Q^T, K^T: [CLOC, T] with head-local d on partitions
            # (M-group mg holds heads 2mg, 2mg+1).
            qT_sb = persist.tile([128, NPAIR, T], bf16)
            kT_sb = persist.tile([128, NPAIR, T], bf16)
            # V per t-tile in natural layout, with a ones column appended per
            # head: vp_sb[:, tt, h, 0:64] = V_h rows, [:, tt, h, 64] = 1.
            # PV stationary [V_h | 1] then yields the softmax denominator in
            # PSUM row 64 for free.
            vp_sb = persist.tile([128, NT, HLOC, D + 1], bf16)
            nc.vector.memset(vp_sb[:, :, :, D : D + 1], 1.0)

            def proj_chunk(w_sb, dst, mg, c):
                """One 512-wide t-chunk of one head pair's projection."""
                ps = ppool.tile([128, 1, 512], f32, tag="pp")
                for kc in range(KC):
                    nc.tensor.matmul(
                        ps[:, 0, :],
                        lhsT=w_sb[:, kc, mg * 128 : (mg + 1) * 128],
                        rhs=xT_sb[:, kc, c * 512 : (c + 1) * 512],
                        start=(kc == 0),
                        stop=(kc == KC - 1),
                    )
                nc.vector.tensor_copy(
                    out=dst[:, mg, c * 512 : (c + 1) * 512], in_=ps[:, 0, :]
                )

            def proj_chunk_units(w_sb, dst, mg, c):
                """Generator form of proj_chunk: yields after every matmul
                so the filler driver can meter it out ~2 matmuls per
                attention group (the PE slack under the exp-paced loop)."""
                ps = ppool.tile([128, 1, 512], f32, tag="pp")
                for kc in range(KC):
                    nc.tensor.matmul(
                        ps[:, 0, :],
                        lhsT=w_sb[:, kc, mg * 128 : (mg + 1) * 128],
                        rhs=xT_sb[:, kc, c * 512 : (c + 1) * 512],
                        start=(kc == 0),
                        stop=(kc == KC - 1),
                    )
                    yield
                nc.vector.tensor_copy(
                    out=dst[:, mg, c * 512 : (c + 1) * 512], in_=ps[:, 0, :]
                )
                yield

            def out_proj_units(tt):
                """Generator: partial = headsN^T @ Wo_loc for t-tile tt."""
                ksl = slice(tt * 128, (tt + 1) * 128)
                psa = ppool.tile([128, 1, 512], f32, tag="pp")
                psb = ppool.tile([128, 1, 512], f32, tag="pp")
                for pp in range(NPAIR):
                    nc.tensor.matmul(
                        psa[:, 0, :],
                        lhsT=headsN[:, pp, ksl],
                        rhs=wo_sb[:, pp, 0:512],
                        start=(pp == 0),
                        stop=(pp == NPAIR - 1),
                    )
                    nc.tensor.matmul(
                        psb[:, 0, :],
                        lhsT=headsN[:, pp, ksl],
                        rhs=wo_sb[:, pp, 512:1024],
                        start=(pp == 0),
                        stop=(pp == NPAIR - 1),
                    )
                    yield
                stg = small.tile([128, 1024], f32, tag="ostg")
                nc.vector.tensor_copy(out=stg[:, 0:512], in_=psa[:, 0, :])
                nc.vector.tensor_copy(out=stg[:, 512:1024], in_=psb[:, 0, :])
                nc.sync.dma_start(out=out.ap()[ksl, :], in_=stg[:])
                yield

            class Filler:
                """Meters queued generators out in ~1-matmul units; the
                emission position of each unit sets its scheduler priority
                so projection/out-proj matmuls slot into the PE slack of
                the exp-paced attention loop instead of forming dense
                blocks that starve the scalar engine.

                CRITICAL: Tile builds the dependency graph in emission
                order, so a consumer emitted before its producer reads
                garbage. require(gen) force-drains up to a generator and
                must be called before emitting anything that reads its
                output."""

                def __init__(self):
                    self.gens = []

                def add(self, gen):
                    self.gens.append(gen)
                    return gen

                def fill(self, n):
                    while n > 0 and self.gens:
                        try:
                            next(self.gens[0])
                            n -= 1
                        except StopIteration:
                            self.gens.pop(0)

                def require(self, gen):
                    while gen in self.gens:
                        self.fill(16)

                def drain(self):
                    while self.gens:
                        self.fill(64)

            def v_proj_pair(tt0):
                # V tiles tt0, tt0+1 in natural [t, c] layout via x^T as the
                # stationary side; both share one ppool tile and one eviction
                ps = ppool.tile([128, 1, 512], f32, tag="pp")
                for half in range(2):
                    tt = tt0 + half
                    for kc in range(KC):
                        nc.tensor.matmul(
                            ps[:, 0, half * CLOC : (half + 1) * CLOC],
                            lhsT=xT_sb[:, kc, tt * 128 : (tt + 1) * 128],
                            rhs=wv_sb[:, kc, :],
                            start=(kc == 0),
                            stop=(kc == KC - 1),
                        )
                nc.vector.tensor_copy(
                    out=vp_sb[:, tt0 : tt0 + 2, :, 0:D],
                    in_=ps[:, 0, :].rearrange(
                        "p (a h d) -> p a h d", a=2, h=HLOC
                    ),
                )

            # headsN[c_lo, pair, t]: plane `pair` holds heads 2p (partitions
            # 0..63) and 2p+1 (64..127) — ready as out-proj stationary tiles.
            headsN = persist.tile([128, NPAIR, T], bf16)

            def attention(pair, tcn, jit_vproj=False, fill=None, rate=2,
                          last=False):
                h0, h1 = 2 * pair, 2 * pair + 1
                tsl = slice(tcn * 512, (tcn + 1) * 512)
                # PV accumulators: one bank per head, rows 0..63 = (P V_h)^T,
                # row 64 = softmax denominator.
                pva = pvpool.tile([D + 1, 512], f32, tag="pva")
                pvb = pvpool.tile([D + 1, 512], f32, tag="pvb")
                # Process T'-tiles in pairs, batching same-shaped matmuls
                # back-to-back — the PE only pipelines (drain under next
                # fill) within runs of same-configuration instructions.
                for g in range(NT // 2):
                    st = (g == 0)
                    sp = (g == NT // 2 - 1)
                    pss, pts = [], []
                    last_s = None
                    for i in range(2):
                        tt = 2 * g + i
                        ksl = slice(tt * 128, (tt + 1) * 128)
                        ps_s = spool.tile([128, 2, 512], f32, tag="sc")
                        pss.append(ps_s)
                        # S^T for both heads (row-packed on the PE)
                        nc.tensor.matmul(
                            ps_s[:, 0, :],
                            lhsT=kT_sb[0:64, pair, ksl],
                            rhs=qT_sb[0:64, pair, tsl],
                            tile_position=(0, 0),
                        )
                        last_s = nc.tensor.matmul(
                            ps_s[:, 1, :],
                            lhsT=kT_sb[64:128, pair, ksl],
                            rhs=qT_sb[64:128, pair, tsl],
                            tile_position=(64, 0),
                        )
                    for i in range(2):
                        pt = ptile.tile([128, 2, 512], bf16, tag="pt")
                        pts.append(pt)
                        nc.scalar.activation(
                            out=pt[:], in_=pss[i][:], func=AF.Exp,
                            scale=0.125,
                        )
                    if jit_vproj:
                        # first consumer of these V tiles: project them
                        # now so the PE fills exp-wait slack
                        v_proj_pair(2 * g)
                    if fill is not None:
                        fill(rate)
                    for i in range(2):
                        tt = 2 * g + i
                        mm = nc.tensor.matmul(
                            pva[:],
                            lhsT=vp_sb[:, tt, h0, :],
                            rhs=pts[i][:, 0, :],
                            start=(st and i == 0), stop=(sp and i == 1),
                        )
                        if i == 0:
                            # keep the two scores groups adjacent on the
                            # PE (same-config runs pipeline; interleaved
                            # configs pay the full isolated matmul cost)
                            add_dep_helper(
                                _inst(mm), _inst(last_s),
                                reason="batch scores before pv",
                            )
                        nc.tensor.matmul(
                            pvb[:],
                            lhsT=vp_sb[:, tt, h1, :],
                            rhs=pts[i][:, 1, :],
                            start=(st and i == 0), stop=(sp and i == 1),
                        )
                # evacuate PSUM quickly so the next chunk's PV can start.
                # h1's rows land on partitions 0..64 (PSUM out starts at the
                # tile base) but must end up on partitions 64..127 of headsN;
                # compute engines can't cross partitions, so DMA-shift the
                # staged copy while the denominator broadcast is in flight.
                stga = small.tile([D + 1, 512], f32, tag="stga")
                nc.vector.tensor_copy(out=stga[:], in_=pva[:])
                stgb = small.tile([D + 1, 512], f32, tag="stgb")
                nc.vector.tensor_copy(out=stgb[:], in_=pvb[:])
                stgb_hi = small.tile([128, 512], f32, tag="stgbh")
                nc.gpsimd.dma_start(
                    out=stgb_hi[64:128, :], in_=stgb[0:D, :]
                )
                # Broadcast each head's denominator row across partitions
                # via a DRAM bounce. Mid-kernel the latency hides under the
                # next phase and sync/gpsimd have spare issue slots; in the
                # last phase the bounce IS the critical path and sync is
                # clogged with out-proj stores, so route everything through
                # the by-then-idle scalar queue.
                rec = small.tile([128, 512], f32, tag="recb")
                den = small.tile([128, 512], f32, tag="den")
                dsc_eng = (nc.scalar, nc.scalar) if last else (nc.sync, nc.sync)
                bc_eng = (nc.scalar, nc.scalar) if last else (nc.gpsimd, nc.gpsimd)
                for j, stg in enumerate((stga, stgb)):
                    dsc = dscratch.tile([1, 512], f32, tag="dsc")
                    dsc_eng[j].dma_start(out=dsc[:], in_=stg[D : D + 1, :])
                    bc_eng[j].dma_start(
                        out=den[j * 64 : (j + 1) * 64, :],
                        in_=dsc[:].to_broadcast([64, 512]),
                    )
                nc.vector.reciprocal_approx_fast(out=rec[:], in_=den[:])
                nc.vector.tensor_mul(
                    out=headsN[0:64, pair, tsl], in0=stga[0:D, :],
                    in1=rec[0:64, :],
                )
                nc.vector.tensor_mul(
                    out=headsN[64:128, pair, tsl], in0=stgb_hi[64:128, :],
                    in1=rec[64:128, :],
                )

            # Emission order IS program order (Tile tracks deps in trace
            # order) and acts as the scheduler's priority tiebreak among
            # ready instructions. The attention chain is exp-paced, so the
            # PE has ~2 spare matmul slots per group; the filler meters the
            # remaining projections and the out-projection through those
            # slots instead of letting them form dense blocks that would
            # starve the scalar engine (or sit in dead phases at the ends).
            filler = Filler()
            proj_chunk(wk_sb, kT_sb, 0, 0)
            proj_chunk(wq_sb, qT_sb, 0, 0)
            proj_chunk(wk_sb, kT_sb, 0, 1)
            # kT chunks 2,3 ride as rate-4 filler inside att(0,0): their
            # evictions are emitted (unit 9 / 18) before the scores of
            # groups 4 / 6 that read them (fill counts 16 / 24).
            filler.add(proj_chunk_units(wk_sb, kT_sb, 0, 2))
            filler.add(proj_chunk_units(wk_sb, kT_sb, 0, 3))
            q01 = filler.add(proj_chunk_units(wq_sb, qT_sb, 0, 1))
            attention(0, 0, jit_vproj=True, fill=filler.fill, rate=4)
            filler.require(q01)
            q02 = filler.add(proj_chunk_units(wq_sb, qT_sb, 0, 2))
            q03 = filler.add(proj_chunk_units(wq_sb, qT_sb, 0, 3))
            for c in range(NTC):
                filler.add(proj_chunk_units(wk_sb, kT_sb, 1, c))
            q10 = filler.add(proj_chunk_units(wq_sb, qT_sb, 1, 0))
            q11 = filler.add(proj_chunk_units(wq_sb, qT_sb, 1, 1))
            attention(0, 1, fill=filler.fill, rate=3)
            filler.require(q02)
            attention(0, 2, fill=filler.fill, rate=3)
            filler.require(q03)
            attention(0, 3, fill=filler.fill, rate=3)
            filler.require(q10)
            q12 = filler.add(proj_chunk_units(wq_sb, qT_sb, 1, 2))
            attention(1, 0, fill=filler.fill)
            filler.require(q11)
            q13 = filler.add(proj_chunk_units(wq_sb, qT_sb, 1, 3))
            for tt in range(0, 4):
                filler.add(out_proj_units(tt))
            attention(1, 1, fill=filler.fill)
            filler.require(q12)
            for tt in range(4, 8):
                filler.add(out_proj_units(tt))
            attention(1, 2, fill=filler.fill)
            filler.require(q13)
            for tt in range(8, 12):
                filler.add(out_proj_units(tt))
            attention(1, 3, fill=filler.fill, last=True)
            for tt in range(12, 16):
                filler.add(out_proj_units(tt))
            filler.drain()

    nc.compile()
    return nc


def _get_program():
    global _cached_nc
    if _cached_nc is None:
        _cached_nc = _build_program()
    return _cached_nc


def kernel(x, Wq, Wk, Wv, Wo):
    global LAST_EXEC_NS
    _ensure_ntff_hook()
    from concourse.bass_utils import run_bass_kernel_spmd

    nc = _get_program()
    bf16 = ml_dtypes.bfloat16

    x = np.asarray(x, dtype=np.float32)
    in_maps = []
    for c in range(NCORES):
        b = c // TPC
        hs = (c % TPC) * HLOC
        xT_c = np.ascontiguousarray(x[b].T).astype(bf16)
        # [HLOC, E, D] -> [E, HLOC*D]
        wq_c = np.ascontiguousarray(
            np.asarray(Wq)[hs : hs + HLOC].transpose(1, 0, 2).reshape(E, CLOC)
        ).astype(bf16)
        wk_c = np.ascontiguousarray(
            np.asarray(Wk)[hs : hs + HLOC].transpose(1, 0, 2).reshape(E, CLOC)
        ).astype(bf16)
        wv_c = np.ascontiguousarray(
            np.asarray(Wv)[hs : hs + HLOC].transpose(1, 0, 2).reshape(E, CLOC)
        ).astype(bf16)
        wo_c = np.ascontiguousarray(
            np.asarray(Wo)[hs * D : (hs + HLOC) * D, :]
        ).astype(bf16)
        in_maps.append(
            {"xT": xT_c, "wq": wq_c, "wk": wk_c, "wv": wv_c, "wo": wo_c}
        )

    trace = bool(os.environ.get("KERNEL_TRACE"))
    res = run_bass_kernel_spmd(nc, in_maps, list(range(NCORES)), trace=trace)
    LAST_EXEC_NS = res.exec_time_ns

    out = np.empty((B, T, E), dtype=np.float32)
    for b in range(B):
        acc = x[b].copy()
        for g in range(TPC):
            acc += res.results[b * TPC + g]["out"]
        out[b] = acc
    return out
